# revision 3
# baseline (speedup 1.0000x reference)
"""Trainium2 Bass kernel for NNConv-style GNN message passing (v2).

Math (edge_attr == ones):
  h   = relu(x @ lin0_w + lin0_b)                      [N, 32]
  W   = (relu(nn_w1[0] + nn_b1) @ nn_w2 + nn_b2).reshape(32, 32)  (constant)
  agg = segment_sum(h[src], dst, N)                    [N, 32]
  out = agg @ W + h @ conv_root + conv_bias            [N, 32]
  score = relu((out[src] * out[dst]) @ lin1_w + lin1_b) @ lin2_w + lin2_b

Mapping to 8 NeuronCores (SPMD):
  * dst-sharded: core c owns dst in [c*6250, (c+1)*6250), 49 blocks of 128
  * node tables (h, out) live ONCE in shared HBM as bf16 rows padded to
    256B; each core writes its shard via ONE indirect-scatter DMA, then a
    tiny AllGather acts as a cross-core barrier (no bulk collectives)
  * src side: dma_gather of 256B rows (1 descriptor/edge); edges split
    into lo/hi streams (src < 31250 vs >=) so row ids fit int16
  * dst side on-chip: phase-1 segment-sum via per-tile one-hot matmuls
    (bf16 iseq); phase-2 expansion via host-precomputed transposed
    one-hots (ohT) as bf16 matmul rhs
  * phase 2 feature-major: A_T via transpose-gather, B_T = out_tab @ ohT
    in PSUM, z = A_T*B_T on DVE, per-tile matmul with lin1, global score
    epilogue
"""
import numpy as np

N_NODES = 50000
N_EDGES = 400000
IN_FEAT = 64
H_DIM = 32
N_CORES = 8
NPC = N_NODES // N_CORES          # 6250
BLOCKS = (NPC + 127) // 128       # 49
LAST_BLK_N = NPC - (BLOCKS - 1) * 128   # 106
P = 128
OP_T = 7                          # tiles per gather op (896 descs, ring-safe)
SPLIT = 31250                     # lo/hi table split (multiple of NPC)


def _bf16():
    import ml_dtypes
    return ml_dtypes.bfloat16


def _prep(x, edge_index):
    """Host-side sharding/slot assembly. Free (untimed)."""
    bf16 = _bf16()
    src = np.asarray(edge_index[0]).astype(np.int64)
    dst = np.asarray(edge_index[1]).astype(np.int64)
    E = src.size
    core = dst // NPC
    d_loc = dst - core * NPC
    blk = d_loc // 128
    half = (src >= SPLIT).astype(np.int64)
    dst_rel = d_loc - blk * 128

    key = (core * BLOCKS + blk) * 2 + half
    counts = np.bincount(key, minlength=N_CORES * BLOCKS * 2)
    counts = counts.reshape(N_CORES, BLOCKS, 2)
    tiles_pos = np.ceil(counts.max(axis=0) / 128).astype(np.int64)
    t_lo = tiles_pos[:, 0]
    t_hi = tiles_pos[:, 1]
    T_LO = int(t_lo.sum())
    T_HI = int(t_hi.sum())
    T_LO_pad = -(-T_LO // OP_T) * OP_T
    T_HI_pad = -(-T_HI // OP_T) * OP_T
    T_ALL = T_LO_pad + T_HI_pad
    n_ops_lo = T_LO_pad // OP_T
    n_ops_hi = T_HI_pad // OP_T
    SLOTS = T_ALL * 128
    lo_base = np.zeros(BLOCKS, np.int64)
    hi_base = np.zeros(BLOCKS, np.int64)
    np.cumsum(t_lo[:-1], out=lo_base[1:])
    np.cumsum(t_hi[:-1], out=hi_base[1:])
    hi_base += T_LO_pad

    order = np.lexsort((np.arange(E), half, blk, core))
    s_src = src[order]
    s_half = half[order]
    s_blk = blk[order]
    s_core = core[order]
    s_dr = dst_rel[order]
    starts = np.zeros(N_CORES * BLOCKS * 2 + 1, np.int64)
    np.cumsum(counts.reshape(-1), out=starts[1:])
    pos = np.arange(E) - starts[(s_core * BLOCKS + s_blk) * 2 + s_half]
    tile_base = np.where(s_half == 0, lo_base[s_blk], hi_base[s_blk])
    slot = tile_base * 128 + pos

    gidx = np.zeros((N_CORES, SLOTS), np.int16)
    dr = np.full((N_CORES, SLOTS), -1.0, np.float32)
    inv = np.full((N_CORES, SLOTS), -1, np.int64)
    row = s_src - s_half * SPLIT
    gidx[s_core, slot] = row.astype(np.int16)
    dr[s_core, slot] = s_dr.astype(np.float32)
    inv[s_core, slot] = order

    n_ops = n_ops_lo + n_ops_hi
    a = gidx.reshape(N_CORES, n_ops, OP_T * 128 // 16, 16).transpose(0, 1, 3, 2)
    w16 = a.transpose(0, 2, 1, 3).reshape(N_CORES, 16, SLOTS // 16)
    gsrc = np.tile(w16, (1, 8, 1))

    dr_pt = dr.reshape(N_CORES, T_ALL, 128).transpose(0, 2, 1)
    dr_pt = np.ascontiguousarray(dr_pt).astype(bf16)

    dmat = np.arange(128, dtype=np.float32)
    ohT = (dr[:, None, :] == dmat[None, :, None]).astype(bf16)

    xs = np.asarray(x, np.float32)
    x_sh = np.zeros((N_CORES, BLOCKS * 128, IN_FEAT), np.float32)
    x_sh[:, :NPC] = xs.reshape(N_CORES, NPC, IN_FEAT)
    xT = np.ascontiguousarray(x_sh.transpose(0, 2, 1)).astype(bf16)

    pp_, bb_ = np.meshgrid(np.arange(128), np.arange(BLOCKS), indexing="ij")
    loc = bb_ * 128 + pp_
    offs_c = np.where(loc[None] < NPC,
                      loc[None] + (np.arange(N_CORES) * NPC)[:, None, None],
                      N_NODES + 10).astype(np.int32)

    spans = []
    for b in range(BLOCKS):
        spans.append((b, 0, int(lo_base[b]), int(t_lo[b])))
    for b in range(BLOCKS):
        spans.append((b, 1, int(hi_base[b]), int(t_hi[b])))

    return dict(T_ALL=T_ALL, T_LO=T_LO, T_HI=T_HI, T_LO_pad=T_LO_pad,
                n_ops_lo=n_ops_lo, n_ops_hi=n_ops_hi, SLOTS=SLOTS,
                gsrc=gsrc, dr_pt=dr_pt, ohT=ohT, xT=xT, offs=offs_c,
                inv=inv, spans=spans)


def _weights(ins, T_ALL):
    bf16 = _bf16()
    f32 = np.float32
    g = {k: np.asarray(v, f32) for k, v in ins.items()}
    v = np.maximum(g["nn_w1"][0] + g["nn_b1"], 0.0)
    W = (v @ g["nn_w2"] + g["nn_b2"]).reshape(H_DIM, H_DIM)
    Wcat = np.concatenate([W, W, g["conv_root"]], 0).astype(bf16)
    w0 = g["lin0_w"].astype(bf16)
    b0c = g["lin0_b"].reshape(H_DIM, 1).astype(f32)
    cb = np.tile(g["conv_bias"][None, :], (P, 1)).astype(f32)
    w1 = g["lin1_w"].astype(bf16)
    b1f = np.tile(np.tile(g["lin1_b"], T_ALL)[None, :], (P, 1)).astype(bf16)
    w2f = np.tile(np.tile(g["lin2_w"][:, 0], T_ALL)[None, :], (P, 1)).astype(bf16)
    b2 = float(g["lin2_b"].reshape(-1)[0])
    iota_f = np.tile(np.arange(P).astype(bf16)[None, :], (P, 1))
    ident = np.tile(np.eye(H_DIM, dtype=np.float32), (4, 1)).astype(bf16)
    return dict(Wcat=Wcat, w0=w0, b0c=b0c, cb=cb, w1=w1, b1f=b1f, w2f=w2f,
                b2=b2, iota_f=iota_f, ident=ident)


def _build(prep_meta, b2_val, k_rep=1):
    import concourse.bacc as bacc
    import concourse.mybir as mybir
    import concourse.tile as tile
    from concourse.library_config import mlp

    T_ALL = prep_meta["T_ALL"]
    n_ops_lo = prep_meta["n_ops_lo"]
    n_ops_hi = prep_meta["n_ops_hi"]
    T_LO = prep_meta["T_LO"]
    T_HI = prep_meta["T_HI"]
    T_LO_pad = prep_meta["T_LO_pad"]
    spans = prep_meta["spans"]
    SLOTS = T_ALL * 128

    f32 = mybir.dt.float32
    bf = mybir.dt.bfloat16
    i16 = mybir.dt.int16
    i32 = mybir.dt.int32

    nc = bacc.Bacc("TRN2", target_bir_lowering=False, debug=False,
                   num_devices=N_CORES)
    dt = nc.dram_tensor
    xT_d = dt("xT", [IN_FEAT, BLOCKS * 128], bf, kind="ExternalInput")
    gsrc_d = dt("gsrc", [128, SLOTS // 16], i16, kind="ExternalInput")
    dr_d = dt("dr_pt", [128, T_ALL], bf, kind="ExternalInput")
    ohT_d = dt("ohT", [128, SLOTS], bf, kind="ExternalInput")
    wc_d = dt("Wcat", [96, 32], bf, kind="ExternalInput")
    w0_d = dt("w0", [64, 32], bf, kind="ExternalInput")
    b0_d = dt("b0c", [32, 1], f32, kind="ExternalInput")
    cb_d = dt("cb", [P, 32], f32, kind="ExternalInput")
    w1_d = dt("w1", [32, 8], bf, kind="ExternalInput")
    b1_d = dt("b1f", [P, T_ALL * 8], bf, kind="ExternalInput")
    w2_d = dt("w2f", [P, T_ALL * 8], bf, kind="ExternalInput")
    io_d = dt("iota_f", [P, P], bf, kind="ExternalInput")
    id_d = dt("ident", [P, 32], bf, kind="ExternalInput")
    b2_d = dt("b2t", [P, 1], f32, kind="ExternalInput")

    h_tab = dt("h_tab", [N_NODES + 16, 128], bf)
    o_tab = dt("o_tab", [N_NODES + 16, 128], bf)
    h_cmp = dt("h_cmp", [N_NODES, 32], bf, addr_space="Shared")
    o_cmp = dt("o_cmp", [N_NODES, 32], bf, addr_space="Shared")
    h_shard = dt("h_shard", [NPC, 32], bf)
    o_shard = dt("o_shard", [NPC, 32], bf)
    sc_d = dt("scores", [P, T_ALL], f32, kind="ExternalOutput")

    groups = [list(range(N_CORES))]
    bypass = mybir.AluOpType.bypass
    add = mybir.AluOpType.add
    mult = mybir.AluOpType.mult
    iseq = mybir.AluOpType.is_equal
    Relu = mybir.ActivationFunctionType.Relu
    X = mybir.AxisListType.X

    tile_info = {}
    span_of = {}
    for (b, hh, t0, ntl) in spans:
        span_of[(b, hh)] = (t0, ntl)
        for j in range(ntl):
            tile_info[t0 + j] = (b, hh, j, j == 0, j == ntl - 1)

    ops = []
    for i in range(n_ops_lo):
        t0 = i * OP_T
        ops.append((0, t0, min(OP_T, T_LO - t0)))
    for i in range(n_ops_hi):
        t0 = i * OP_T
        ops.append((1, T_LO_pad + t0, min(OP_T, T_HI - t0)))

    with tile.TileContext(nc) as tc:
        with tc.tile_pool(name="persist", bufs=1) as pp:
            nc.gpsimd.load_library(mlp)
            gsrc_sb = pp.tile([128, SLOTS // 16], i16)
            dr_sb = pp.tile([128, T_ALL], bf)
            wc_sb = pp.tile([96, 32], bf)
            w0_sb = pp.tile([64, 32], bf)
            b0_sb = pp.tile([32, 1], f32)
            cb_sb = pp.tile([P, 32], f32)
            w1_sb = pp.tile([32, 8], bf)
            b1_sb = pp.tile([P, T_ALL * 8], bf)
            w2_sb = pp.tile([P, T_ALL * 8], bf)
            io_sb = pp.tile([P, P], bf)
            id_sb = pp.tile([P, 32], bf)
            b2_sb = pp.tile([P, 1], f32)
            for sb, d in [(gsrc_sb, gsrc_d), (dr_sb, dr_d), (wc_sb, wc_d),
                          (w0_sb, w0_d), (b0_sb, b0_d), (cb_sb, cb_d),
                          (w1_sb, w1_d), (b1_sb, b1_d), (w2_sb, w2_d),
                          (io_sb, io_d), (id_sb, id_d), (b2_sb, b2_d)]:
                nc.sync.dma_start(out=sb[:], in_=d[:])

            hxT = pp.tile([96, BLOCKS * 128], bf)
            h_all = pp.tile([128, BLOCKS, 32], bf)
            out_tab = pp.tile([128, BLOCKS, 32], bf)
            m_sb = pp.tile([128, T_ALL * 8], f32)
            sc_sb = pp.tile([128, T_ALL], f32)
            s1 = pp.tile([128, T_ALL * 8], bf)

            for _rep in range(k_rep):
                nc.vector.memset(hxT[:], 0.0)
                nc.vector.memset(m_sb[:], 0.0)

                # ---------------- phase 0: h = relu(x @ w0 + b0) ----------
                with (
                    tc.tile_pool(name="p0", bufs=3) as p0,
                    tc.tile_pool(name="p0p", bufs=3, space="PSUM") as p0p,
                ):
                    for b in range(BLOCKS):
                        xt = p0.tile([IN_FEAT, P], bf, tag="xt")
                        nc.sync.dma_start(
                            out=xt[:], in_=xT_d[:, b * 128:(b + 1) * 128])
                        ps_hT = p0p.tile([32, P], f32, tag="hT")
                        nc.tensor.matmul(out=ps_hT[:], lhsT=w0_sb[:],
                                         rhs=xt[:], start=True, stop=True)
                        nc.scalar.activation(
                            out=hxT[64:96, b * 128:(b + 1) * 128],
                            in_=ps_hT[:], func=Relu, bias=b0_sb[:])
                        ps_h = p0p.tile([P, 32], bf, tag="h")
                        nc.tensor.transpose(
                            out=ps_h[:],
                            in_=hxT[64:96, b * 128:(b + 1) * 128],
                            identity=id_sb[64:96, :])
                        nc.vector.tensor_copy(out=h_all[:, b, :], in_=ps_h[:])
                        n = 128 if b < BLOCKS - 1 else LAST_BLK_N
                        nc.sync.dma_start(
                            out=h_shard[b * 128:b * 128 + n, :],
                            in_=h_all[:n, b, :])
                nc.gpsimd.collective_compute(
                    "AllGather", bypass, groups,
                    ins=[h_shard[:]], outs=[h_cmp[:]])
                with tc.tile_pool(name="rp1_%d" % _rep, bufs=3) as rpool:
                    CH, QN = 8192, 64
                    for k in range(6):
                        rp = rpool.tile([128, QN, 32], bf, tag="rp")
                        nc.sync.dma_start(
                            out=rp[:],
                            in_=h_cmp[k * CH:(k + 1) * CH, :]
                            .rearrange("(p q) f -> p q f", q=QN))
                        nc.sync.dma_start(
                            out=h_tab[k * CH:(k + 1) * CH, 0:32]
                            .rearrange("(p q) f -> p q f", q=QN),
                            in_=rp[:])
                    rpt = rpool.tile([106, 8, 32], bf, tag="rpt")
                    nc.sync.dma_start(
                        out=rpt[:],
                        in_=h_cmp[6 * CH:N_NODES, :]
                        .rearrange("(p q) f -> p q f", q=8))
                    nc.sync.dma_start(
                        out=h_tab[6 * CH:N_NODES, 0:32]
                        .rearrange("(p q) f -> p q f", q=8),
                        in_=rpt[:])

                # ------- phase 1: gather h[src], one-hot aggregate --------
                with (
                    tc.tile_pool(name="p1", bufs=3) as p1,
                    tc.tile_pool(name="p1p", bufs=4, space="PSUM") as p1p,
                ):
                    ps_agg = {}
                    for half, t0, nreal in ops:
                        base = 0 if half == 0 else SPLIT
                        nrow = SPLIT if half == 0 else N_NODES - SPLIT
                        gd = p1.tile([P, OP_T, 128], bf, tag="gd")
                        nc.gpsimd.dma_gather(
                            gd[:], h_tab[base:base + nrow, :],
                            gsrc_sb[:, t0 * 8:(t0 + OP_T) * 8],
                            OP_T * 128, OP_T * 128, 128)
                        oh = p1.tile([P, OP_T, 128], bf, tag="oh")
                        nc.vector.tensor_tensor(
                            out=oh[:],
                            in0=dr_sb[:, t0:t0 + OP_T]
                            .rearrange("p (t o) -> p t o", o=1)
                            .to_broadcast([P, OP_T, P]),
                            in1=io_sb[:].rearrange("p (o f) -> p o f", o=1)
                            .to_broadcast([P, OP_T, P]),
                            op=iseq)
                        for i in range(nreal):
                            t = t0 + i
                            info = tile_info.get(t)
                            if info is None:
                                continue
                            b, hh, j, first, last = info
                            if first:
                                agg_t = p1p.tile([32, P], f32, tag="agg")
                                ps_agg[(b, hh)] = agg_t
                            nc.tensor.matmul(
                                out=ps_agg[(b, hh)][:],
                                lhsT=gd[:, i, 0:32], rhs=oh[:, i, :],
                                start=first, stop=last)
                            if last:
                                nc.scalar.copy(
                                    out=hxT[hh * 32:hh * 32 + 32,
                                            b * 128:(b + 1) * 128],
                                    in_=ps_agg.pop((b, hh))[:])
                    for b in range(BLOCKS):
                        ps_o = p1p.tile([P, 32], f32, tag="po")
                        nc.tensor.matmul(
                            out=ps_o[:], lhsT=hxT[:, b * 128:(b + 1) * 128],
                            rhs=wc_sb[:], start=True, stop=True)
                        nc.vector.tensor_tensor(
                            out=out_tab[:, b, :], in0=ps_o[:], in1=cb_sb[:],
                            op=add)
                        n = 128 if b < BLOCKS - 1 else LAST_BLK_N
                        nc.sync.dma_start(
                            out=o_shard[b * 128:b * 128 + n, :],
                            in_=out_tab[:n, b, :])
                nc.gpsimd.collective_compute(
                    "AllGather", bypass, groups,
                    ins=[o_shard[:]], outs=[o_cmp[:]])
                with tc.tile_pool(name="rp2_%d" % _rep, bufs=3) as rpool2:
                    CH, QN = 8192, 64
                    for k in range(6):
                        rp2 = rpool2.tile([128, QN, 32], bf, tag="rp")
                        nc.sync.dma_start(
                            out=rp2[:],
                            in_=o_cmp[k * CH:(k + 1) * CH, :]
                            .rearrange("(p q) f -> p q f", q=QN))
                        nc.sync.dma_start(
                            out=o_tab[k * CH:(k + 1) * CH, 0:32]
                            .rearrange("(p q) f -> p q f", q=QN),
                            in_=rp2[:])
                    rpt2 = rpool2.tile([106, 8, 32], bf, tag="rpt")
                    nc.sync.dma_start(
                        out=rpt2[:],
                        in_=o_cmp[6 * CH:N_NODES, :]
                        .rearrange("(p q) f -> p q f", q=8))
                    nc.sync.dma_start(
                        out=o_tab[6 * CH:N_NODES, 0:32]
                        .rearrange("(p q) f -> p q f", q=8),
                        in_=rpt2[:])

                # ---------------- phase 2: edge scores --------------------
                with (
                    tc.tile_pool(name="p2", bufs=3) as p2,
                    tc.tile_pool(name="p2p", bufs=4, space="PSUM") as p2p,
                ):
                    ps_m = {}
                    for half, t0, nreal in ops:
                        base = 0 if half == 0 else SPLIT
                        nrow = SPLIT if half == 0 else N_NODES - SPLIT
                        gdT = p2.tile([P, 1, OP_T * 128], bf, tag="gdT")
                        nc.gpsimd.dma_gather(
                            gdT[:], o_tab[base:base + nrow, :],
                            gsrc_sb[:, t0 * 8:(t0 + OP_T) * 8],
                            OP_T * 128, OP_T * 128, 128, transpose=True)
                        ohT = p2.tile([P, OP_T * 128], bf, tag="ohT")
                        nc.sync.dma_start(
                            out=ohT[:],
                            in_=ohT_d[:, t0 * 128:(t0 + OP_T) * 128])
                        zT = p2.tile([32, OP_T * 128], bf, tag="zT")
                        real_w = nreal * 128
                        for c0 in range(0, real_w, 448):
                            w = min(448, real_w - c0)
                            psB = p2p.tile([32, 448], f32, tag="B")
                            cc = c0
                            while cc < c0 + w:
                                t = t0 + cc // 128
                                b, hh, j, first, last = tile_info[t]
                                st0, ntl = span_of[(b, hh)]
                                seg_end = min(c0 + w, (st0 + ntl - t0) * 128)
                                nc.tensor.matmul(
                                    out=psB[:, cc - c0:seg_end - c0],
                                    lhsT=out_tab[:, b, :],
                                    rhs=ohT[:, cc:seg_end],
                                    start=True, stop=True)
                                cc = seg_end
                            nc.vector.tensor_tensor(
                                out=zT[:, c0:c0 + w],
                                in0=gdT[0:32, 0, c0:c0 + w],
                                in1=psB[:, 0:w], op=mult)
                        for i in range(nreal):
                            t = t0 + i
                            b, hh, j, first, last = tile_info[t]
                            key = (b, hh)
                            if first:
                                st0, ntl = span_of[key]
                                m_t = p2p.tile([P, 8 * ntl], f32, tag="m")
                                ps_m[key] = m_t
                            pm = ps_m[key]
                            nc.tensor.matmul(
                                out=pm[:, j * 8:(j + 1) * 8],
                                lhsT=zT[:, i * 128:(i + 1) * 128],
                                rhs=w1_sb[:], start=True, stop=True)
                            if last:
                                st0, ntl = span_of[key]
                                nc.vector.tensor_copy(
                                    out=m_sb[:, st0 * 8:(st0 + ntl) * 8],
                                    in_=pm[:])
                                ps_m.pop(key)
                    nc.vector.tensor_tensor(out=s1[:], in0=m_sb[:],
                                            in1=b1_sb[:], op=add)
                    nc.scalar.activation(out=s1[:], in_=s1[:], func=Relu)
                    nc.vector.tensor_tensor(out=s1[:], in0=s1[:],
                                            in1=w2_sb[:], op=mult)
                    nc.vector.reduce_sum(
                        out=sc_sb[:],
                        in_=s1[:].rearrange("p (t e) -> p t e", e=8),
                        axis=X)
                    nc.vector.tensor_tensor(
                        out=sc_sb[:], in0=sc_sb[:],
                        in1=b2_sb[:].to_broadcast([P, T_ALL]), op=add)
                    nc.sync.dma_start(out=sc_d[:], in_=sc_sb[:])
    nc.compile()
    return nc


def _in_maps(prep, wts):
    maps = []
    for c in range(N_CORES):
        maps.append({
            "xT": prep["xT"][c], "gsrc": prep["gsrc"][c],
            "dr_pt": prep["dr_pt"][c], "ohT": prep["ohT"][c],
            "Wcat": wts["Wcat"], "w0": wts["w0"], "b0c": wts["b0c"],
            "cb": wts["cb"], "w1": wts["w1"], "b1f": wts["b1f"],
            "w2f": wts["w2f"], "iota_f": wts["iota_f"],
            "ident": wts["ident"],
            "b2t": np.full((128, 1), wts["b2"], np.float32),
        })
    return maps


def _assemble(results, prep):
    scores = np.empty(N_EDGES, np.float32)
    for c in range(N_CORES):
        flat = results[c]["scores"].T.reshape(-1)
        inv = prep["inv"][c]
        m = inv >= 0
        scores[inv[m]] = flat[m]
    return scores


def kernel(**inputs):
    from concourse.bass_utils import run_bass_kernel_spmd
    prep = _prep(inputs["x"], inputs["edge_index"])
    wts = _weights(inputs, prep["T_ALL"])
    nc = _build(prep, wts["b2"], k_rep=1)
    res = run_bass_kernel_spmd(nc, _in_maps(prep, wts),
                               list(range(N_CORES)))
    return _assemble(res.results, prep)
